# revision 5
# baseline (speedup 1.0000x reference)
"""Trainium2 Bass kernel for upsample_conv_2d (conv_transpose stride-2 3x3 +
4x4 FIR + bias), data-parallel over batch on 8 NeuronCores.

Algorithm (per core = one batch image):

Stage 1 (PE): phase-decomposed conv_transpose. y[2R+pa, 2S+pb] =
  sum_{i,j,ci} w[pa+2i, pb+2j][ci,co] * x[ci, R-1+pa+i, S-1+pb+j]
-> 9 channel-contraction taps total across the 4 phases (vs 36 for the
fully-composed kernel). Weights are pre-scaled by 1/16 (the FIR per-axis
1/4 gains) and bias/64 is folded in during the PSUM->SBUF drain (ACT),
which also casts to bf16. Phase tiles Yp[pa] are [128, 66, 132] with the
two column phases packed side by side and a bias/64 pad frame so the FIR
boundary handling is exact.

Stage 2 (GpSimd + DVE): the 4x4 FIR = outer((1,3,3,1),(1,3,3,1))/16 on the
2x-upsampled grid, evaluated as three box-filter adds per axis directly in
phase space (bf16, DVE 2x mode), in chunks of 16 output rows:
  C1[m] = y[m] + y[m+1]; C2[m] = C1[m] + C1[m+1]; V[A] = C2[A-1] + C2[A]
then the same cascade over columns; the final add writes fp32 directly
into the interleaved output staging tile.

Issue order: stage-1 iterates row-groups outer / phases inner (edge strips
first) so stage-2 chunks become runnable early; stage-2(cob=0) is emitted
interleaved with stage-1(cob=1) to keep all engines busy.
"""

import json

import numpy as np

import concourse.bass as bass
import concourse.mybir as mybir
import concourse.tile as tile
from concourse.bass_utils import run_bass_kernel_spmd

# ---------------------------------------------------------------------------
# BIR post-pass: this walrus build rejects instructions carrying more than one
# sem wait (e.g. Tile's kernel-tail Drain gets 3). Hoist extras into
# standalone EventSemaphore instructions right before the owner.
# ---------------------------------------------------------------------------
_MAX_WAITS = 1


def _split_waits(j: dict) -> dict:
    for fn in j.get("functions", []):
        for blk in fn.get("blocks", []):
            insts = blk.get("instructions")
            if not insts:
                continue
            out = []
            for inst in insts:
                si = inst.get("sync_info") or {}
                waits = si.get("on_wait") or []
                if len(waits) > _MAX_WAITS:
                    for k, w in enumerate(waits[_MAX_WAITS:]):
                        out.append(
                            {
                                "debug": inst.get("debug", 0),
                                "engine": inst["engine"],
                                "ins": [],
                                "name": f"{inst['name']}-wsplit{k}",
                                "opcode": "EventSemaphore",
                                "outs": [],
                                "sync_info": {"on_update": [], "on_wait": [w]},
                            }
                        )
                    si["on_wait"] = waits[:_MAX_WAITS]
                out.append(inst)
            blk["instructions"] = out
    return j


_orig_to_json_bytes = bass.Bass.to_json_bytes


def _patched_to_json_bytes(self):
    return json.dumps(_split_waits(json.loads(_orig_to_json_bytes(self)))).encode()


bass.Bass.to_json_bytes = _patched_to_json_bytes

# ---------------------------------------------------------------------------
# Problem constants (hardcoded; kernel.py must be self-contained)
# ---------------------------------------------------------------------------
N, C, H, W = 8, 256, 64, 64
OH, OW = 2 * H, 2 * W
N_CORES = 8
F32 = mybir.dt.float32
F32R = mybir.dt.float32r
BF16 = mybir.dt.bfloat16
IDENT = mybir.ActivationFunctionType.Identity

_PHASES = [(0, 0), (0, 1), (1, 0), (1, 1)]


def _taps(pa, pb):
    ii = (0, 1) if pa == 0 else (0,)
    jj = (0, 1) if pb == 0 else (0,)
    return [(i, j) for i in ii for j in jj]


_WBLOCKS = []
for pa, pb in _PHASES:
    for i, j in _taps(pa, pb):
        for cib in range(2):
            for cob in range(2):
                _WBLOCKS.append((pa, pb, i, j, cib, cob))
_WIDX = {k: n for n, k in enumerate(_WBLOCKS)}
NW = len(_WBLOCKS)  # 36


def _stage1_weights(w: np.ndarray) -> np.ndarray:
    """[256,256,3,3] -> lhsT [128 ci, NW, 128 co], scaled by 1/16."""
    Wm = np.zeros((128, NW, 128), dtype=np.float32)
    for n, (pa, pb, i, j, cib, cob) in enumerate(_WBLOCKS):
        blk = w[
            cob * 128 : (cob + 1) * 128, cib * 128 : (cib + 1) * 128, pa + 2 * i, pb + 2 * j
        ]  # [co, ci]
        Wm[:, n, :] = blk.T / 16.0
    return Wm


def build_nc(reps: int = 1) -> bass.Bass:
    nc = bass.Bass("TRN2", target_bir_lowering=False, debug=False)
    x_d = nc.dram_tensor("x", [C, H + 2, W + 2], BF16, kind="ExternalInput").ap()
    w_d = nc.dram_tensor("w", [128, NW * 128], BF16, kind="ExternalInput").ap()
    b_d = nc.dram_tensor("bias", [2, 128], F32, kind="ExternalInput").ap()
    out_d = nc.dram_tensor("out", [C, OH, OW], F32, kind="ExternalOutput").ap()

    xb = x_d.rearrange("(b p) h w -> b p h w", p=128)
    wb = w_d.rearrange("p (a b) -> p a b", b=128)

    with tile.TileContext(nc) as tc:
        with (
            tc.tile_pool(name="const", bufs=1) as cpool,
            tc.tile_pool(name="ypers", bufs=1) as ypool,
            tc.tile_pool(name="psum", bufs=3, space="PSUM") as ppool,
            tc.tile_pool(name="pedge", bufs=2, space="PSUM") as epool,
            tc.tile_pool(name="s2", bufs=3) as spool,
            tc.tile_pool(name="c1", bufs=6) as c1pool,
            tc.tile_pool(name="outs", bufs=3) as opool,
        ):
            # split input DMAs into bands so PE can start early
            wt = cpool.tile([128, NW, 128], BF16)
            for h in range(2):
                nc.sync.dma_start(wt[:, h * 18 : h * 18 + 18, :], wb[:, h * 18 : h * 18 + 18, :])
            bt = cpool.tile([128, 2], F32)
            nc.sync.dma_start(bt[:], b_d.rearrange("b p -> p b"))
            zt = cpool.tile([128, 132], F32)
            nc.vector.memset(zt[:], 0.0)

            xpad = [cpool.tile([128, H + 2, W + 2], BF16, name=f"xp{i}") for i in range(2)]
            for cib in range(2):
                for r0, r1 in ((0, 24), (24, 48), (48, 66)):
                    nc.sync.dma_start(
                        xpad[cib][:, r0:r1, :], xb[cib][:, r0:r1, :]
                    )

            # persistent Y phase tiles, frames pre-filled with bias/64
            Yp = {}
            for cob in range(2):
                for pa in range(2):
                    t = ypool.tile([128, 66, 132], BF16, name=f"Y{cob}{pa}")
                    Yp[(cob, pa)] = t
                    bias_ap = bt[:, cob : cob + 1]
                    frame_rows = [65] if pa == 0 else [0, 65]
                    for fr in frame_rows:
                        nc.scalar.activation(
                            t[:, fr, :], zt[:], IDENT, bias=bias_ap, scale=1.0
                        )
                    for fc in (65, 66, 131):
                        nc.scalar.activation(
                            t[:, :, fc], zt[:, 0:66], IDENT, bias=bias_ap, scale=1.0
                        )

            def s1_edges(cob):
                """Edge col strips (S=64 for pb=0 phases) + row remainders
                (R=64 for pa=0 phases), all accumulated in one psum bank."""
                bias_ap = bt[:, cob : cob + 1]
                pe = epool.tile([128, 512], F32, tag="pe", name="pe")
                off = 0
                drains = []
                for pa, pb in _PHASES:
                    taps = _taps(pa, pb)
                    nR = 65 if pa == 0 else 64
                    t0 = 0 if pa == 0 else 1
                    u0 = 0 if pb == 0 else 67
                    yt = Yp[(cob, pa)]
                    acc = [(i, j, cib) for (i, j) in taps for cib in range(2)]
                    if pb == 0:  # col strip S=64, rows 0..nR-1
                        for st, (i, j, cib) in enumerate(acc):
                            lhsT = wt[:, _WIDX[(pa, pb, i, j, cib, cob)], :]
                            rhs = xpad[cib][:, pa + i : pa + i + nR, 64 + pb + j]
                            nc.tensor.matmul(
                                pe[:, off : off + nR],
                                lhsT,
                                rhs,
                                start=(st == 0),
                                stop=(st == len(acc) - 1),
                            )
                        drains.append((yt[:, t0 : t0 + nR, u0 + 64], pe[:, off : off + nR]))
                        off += nR
                    if pa == 0:  # row remainder R=64, cols 0..63
                        for st, (i, j, cib) in enumerate(acc):
                            lhsT = wt[:, _WIDX[(pa, pb, i, j, cib, cob)], :]
                            rhs = xpad[cib][:, 64 + pa + i, pb + j : pb + j + 64]
                            nc.tensor.matmul(
                                pe[:, off : off + 64],
                                lhsT,
                                rhs,
                                start=(st == 0),
                                stop=(st == len(acc) - 1),
                            )
                        drains.append((yt[:, t0 + 64, u0 : u0 + 64], pe[:, off : off + 64]))
                        off += 64
                for dst, src in drains:
                    nc.scalar.activation(dst, src, IDENT, bias=bias_ap, scale=1.0)

            def s1_rowgroup(cob, rg):
                """Main-grid rows rg*16..rg*16+15, cols 0..63, all 4 phases."""
                bias_ap = bt[:, cob : cob + 1]
                R0 = rg * 16
                for pa, pb in _PHASES:
                    taps = _taps(pa, pb)
                    t0 = 0 if pa == 0 else 1
                    u0 = 0 if pb == 0 else 67
                    yt = Yp[(cob, pa)]
                    ps = ppool.tile([128, 16, 64], F32, tag="ps", name="ps")
                    for sub in range(2):
                        Rs = R0 + sub * 8
                        acc = [(i, j, cib) for (i, j) in taps for cib in range(2)]
                        for st, (i, j, cib) in enumerate(acc):
                            lhsT = wt[:, _WIDX[(pa, pb, i, j, cib, cob)], :]
                            rhs = xpad[cib][
                                :, Rs + pa + i : Rs + pa + i + 8, pb + j : pb + j + 64
                            ]
                            nc.tensor.matmul(
                                ps[:, sub * 8 : sub * 8 + 8, :],
                                lhsT,
                                rhs,
                                start=(st == 0),
                                stop=(st == len(acc) - 1),
                            )
                    nc.scalar.activation(
                        yt[:, t0 + R0 : t0 + R0 + 16, u0 : u0 + 64],
                        ps[:],
                        IDENT,
                        bias=bias_ap,
                        scale=1.0,
                    )

            def s2_chunk(cob, k):
                """16 output rows 16k..16k+15."""
                y0 = Yp[(cob, 0)]
                y1 = Yp[(cob, 1)]
                r = 8 * k
                c1e = c1pool.tile([128, 9, 132], BF16, tag="c1e", name="c1e")
                c1o = spool.tile([128, 9, 132], BF16, tag="c1o", name="c1o")
                nc.gpsimd.tensor_add(c1e[:], y0[:, r : r + 9, :], y1[:, r + 1 : r + 10, :])
                nc.vector.tensor_add(c1o[:], y1[:, r : r + 9, :], y0[:, r : r + 9, :])
                c2e = spool.tile([128, 8, 132], BF16, tag="c2e", name="c2e")
                c2o = spool.tile([128, 9, 132], BF16, tag="c2o", name="c2o")
                nc.vector.tensor_add(c2e[:], c1e[:, 0:8, :], c1o[:, 1:9, :])
                nc.vector.tensor_add(c2o[:], c1o[:], c1e[:])
                # V packed: rows 0..7 = even out rows (ra=0), 8..15 = odd
                vv = spool.tile([128, 16, 132], BF16, tag="vv", name="vv")
                nc.vector.tensor_add(vv[:, 0:8, :], c2o[:, 0:8, :], c2e[:])
                nc.vector.tensor_add(vv[:, 8:16, :], c2e[:], c2o[:, 1:9, :])
                d1e = spool.tile([128, 16, 65], BF16, tag="d1e", name="d1e")
                d1o = spool.tile([128, 16, 65], BF16, tag="d1o", name="d1o")
                nc.vector.tensor_add(d1e[:], vv[:, :, 0:65], vv[:, :, 67:132])
                nc.vector.tensor_add(d1o[:], vv[:, :, 66:131], vv[:, :, 0:65])
                d2e = spool.tile([128, 16, 64], BF16, tag="d2e", name="d2e")
                d2o = spool.tile([128, 16, 65], BF16, tag="d2o", name="d2o")
                nc.vector.tensor_add(d2e[:], d1e[:, :, 0:64], d1o[:, :, 1:65])
                nc.vector.tensor_add(d2o[:], d1o[:], d1e[:])
                # Finals on GpSimd, writing fp32 interleaved straight into the
                # DMA staging tile: keeps the slow engine off the DVE critical
                # path (tail work feeding the DMA only).
                og = opool.tile([128, 16, 128], F32, tag="og", name="og")
                ov = og.rearrange("p (q a) (s b) -> p q a s b", a=2, b=2)
                for rb in range(2):
                    # in-order dims (ra, q, S) -> out AP [2ra, 8q, 64S]
                    dsrc0 = d2o[:, :, 0:64] if rb == 0 else d2e[:, :, :]
                    dsrc1 = d2e[:, :, :] if rb == 0 else d2o[:, :, 1:65]
                    dst_ap = ov.rearrange("p q a s b -> p a q s b")[:, :, :, :, rb]
                    nc.gpsimd.tensor_add(
                        dst_ap,
                        dsrc0.rearrange("p (a q) s -> p a q s", a=2),
                        dsrc1.rearrange("p (a q) s -> p a q s", a=2),
                    )
                dst = out_d[cob * 128 : (cob + 1) * 128, 16 * k : 16 * k + 16, :]
                nc.sync.dma_start(dst, og[:])

            def body():
                # interleave: emit each stage-2 chunk right after the last
                # stage-1 row-group its Y rows depend on has been issued.
                for cob in range(2):
                    s1_edges(cob)
                    s1_rowgroup(cob, 0)
                    s1_rowgroup(cob, 1)
                    s2_chunk(cob, 0)
                    s2_chunk(cob, 1)
                    s2_chunk(cob, 2)
                    s1_rowgroup(cob, 2)
                    s2_chunk(cob, 3)
                    s2_chunk(cob, 4)
                    s1_rowgroup(cob, 3)
                    s2_chunk(cob, 5)
                    s2_chunk(cob, 6)
                    s2_chunk(cob, 7)

            if reps == 1:
                body()
            else:
                with tc.For_i(0, reps):
                    body()
    return nc


_CACHED_NC = {}


def _get_nc(reps: int = 1) -> bass.Bass:
    if reps not in _CACHED_NC:
        _CACHED_NC[reps] = build_nc(reps)
    return _CACHED_NC[reps]


def _prep(x, weight, bias):
    import ml_dtypes

    Wm = _stage1_weights(np.asarray(weight, dtype=np.float32))
    b2 = np.ascontiguousarray(
        (np.asarray(bias, dtype=np.float32) / 64.0).reshape(2, 128)
    )
    xs = np.pad(
        np.asarray(x, dtype=np.float32), ((0, 0), (0, 0), (1, 1), (1, 1))
    )
    return (
        xs.astype(ml_dtypes.bfloat16),
        Wm.reshape(128, -1).astype(ml_dtypes.bfloat16),
        b2,
    )


def _run(x, weight, bias, reps: int = 1):
    xs, Wm, b2 = _prep(x, weight, bias)
    nc = _get_nc(reps)
    in_maps = [{"x": xs[i], "w": Wm, "bias": b2} for i in range(N_CORES)]
    res = run_bass_kernel_spmd(nc, in_maps, list(range(N_CORES)))
    return np.stack([res.results[i]["out"] for i in range(N_CORES)])


def kernel(x, weight, bias):
    return _run(x, weight, bias, reps=1)
